# revision 7
# baseline (speedup 1.0000x reference)
"""CoGOL ordinal-logistic loss on 8 Trainium2 NeuronCores.

Math (per sample, target t in [1,64], logits x[0..62], only x[0..61] used):
  -loss_i = P_i - G_i - X_i + ln2*[t>=2]
with
  P_i = sum_{j=0}^{61} softplus(x_ij)
  G_i = softplus(x_{i,t-2})          (only when 2 <= t <= 63)
  X_i = sum_{j=t-1}^{61} x_ij
final = (1/B) * sum_i (-loss_i) + a/2*sum(w^2) + b/2*sum(d[1:]^2).

All target-dependent terms collapse through class-conditional column sums:
  Hx[j,c] = sum_{i: t_i=c} x_ij,  Hs[j,c] = sum_{i: t_i=c} softplus(x)_ij
computed as one-hot matmuls on the (otherwise idle) tensor engine, PSUM-
accumulated over 512 row-blocks of 128. A constant 126x64 coefficient
matrix then reduces Hx/Hs to the scalar:
  sum coefX*Hx = -X_total, sum coefS*Hs = P_total - G_total.

Per-element engine work: ONE Softplus activation pass (scalar engine) and
ONE is_equal pass building the one-hot (vector engine). The f32->bf16
conversion of x rides the HBM load for free (gpsimd SWDGE casting DMA),
so DMA (16.5 MB/core of f32 logits) is the bottleneck.

Sharding: batch split 8 ways (65536 rows/core); weights flat-split 8 ways;
deltas[1:] to core 0 only. Each core emits one partial scalar; host sums.
"""

import sys

sys.path.insert(0, "/opt/trn_rl_repo")

import numpy as np

ALPHA = 0.01
BETA = 0.05
B = 524288
KM1 = 63
NCORES = 8
BC = B // NCORES              # 65536 rows per core
RTOT = BC // 128              # 512 rows per partition
NCH = 8                       # chunks per core
R = RTOT // NCH               # 64 rows per partition per chunk
WPER = (3 * 512 * 512) // NCORES  # 98304 weight elements per core
LN2 = 0.6931471805599453

_PROG = None


def _coef() -> np.ndarray:
    """[126, 64] coefficient matrix: rows 0..62 hit Hx (x columns), rows
    63..125 hit Hs (softplus columns); class c = col index + 1."""
    jj = np.arange(63)[:, None]
    cc = np.arange(1, 65)[None, :]
    used = jj <= 61
    coef = np.zeros((126, 64), np.float32)
    coef[0:63] = -((jj >= cc - 1) & used).astype(np.float32)
    coef[63:126] = used.astype(np.float32) - ((jj == cc - 2) & used).astype(np.float32)
    return coef


def _build():
    import concourse.bacc as bacc
    import concourse.tile as tile
    from concourse import mybir

    # Exp and Ln both live in the "natural_log_exp_and_others" ACT table set,
    # but the table-load inserter picks the first set containing each func,
    # which ping-pongs between two sets (a ~1.3us reload per activation).
    # Blank every other set (order preserved, so set ids stay valid) to force
    # a single resident table.
    if not getattr(bacc, "_act_tables_pinned", False):
        _orig_get = bacc.get_activation_tables

        def _pinned(arch, _orig=_orig_get):
            tabs = _orig(arch)
            keep = "natural_log_exp_and_others"
            return {k: (v if k == keep else set()) for k, v in tabs.items()}

        bacc.get_activation_tables = _pinned
        bacc._act_tables_pinned = True

    f32 = mybir.dt.float32
    bf16 = mybir.dt.bfloat16
    i32 = mybir.dt.int32
    Alu = mybir.AluOpType
    Act = mybir.ActivationFunctionType

    nc = bacc.Bacc("TRN2", target_bir_lowering=False, debug=False, num_devices=NCORES)

    logits = nc.dram_tensor("logits", [BC, KM1], f32, kind="ExternalInput")
    targets = nc.dram_tensor("targets", [BC], f32, kind="ExternalInput")
    wts = nc.dram_tensor("wts", [WPER], f32, kind="ExternalInput")
    dls = nc.dram_tensor("dls", [192], f32, kind="ExternalInput")
    coef = nc.dram_tensor("coef", [126, 64], f32, kind="ExternalInput")
    out = nc.dram_tensor("out", [1, 1], f32, kind="ExternalOutput")

    with tile.TileContext(nc) as tc:
        with (
            tc.tile_pool(name="const", bufs=1) as cpool,
            tc.tile_pool(name="xs", bufs=3) as xpool,
            tc.tile_pool(name="ex", bufs=3) as epool,
            tc.tile_pool(name="oh", bufs=3) as opool,
            tc.tile_pool(name="fin", bufs=1) as fpool,
            tc.tile_pool(name="ps", bufs=1, space="PSUM") as ppool,
            tc.tile_pool(name="ps2", bufs=1, space="PSUM") as p2pool,
        ):
            # constants: class iota 1..64, ones column, coef matrix, targets
            iota_i = cpool.tile([128, 64], i32)
            nc.gpsimd.iota(iota_i[:], pattern=[[1, 64]], base=1,
                           channel_multiplier=0)
            iota_f = cpool.tile([128, 64], f32)
            nc.vector.tensor_copy(iota_f[:], iota_i[:])
            ones = cpool.tile([128, 1], f32)
            nc.vector.memset(ones[:], 1.0)
            coeft = cpool.tile([126, 64], f32)
            nc.sync.dma_start(coeft[:], coef.ap())
            # targets in per-partition layout: tload[p, r] = targets[p*512+r]
            tload = cpool.tile([128, RTOT], f32)
            nc.sync.dma_start(
                tload[:], targets.ap().rearrange("(p r) -> p r", p=128)
            )

            psum = ppool.tile([126, 64], f32)
            lg = logits.ap().rearrange("(p r) c -> p r c", p=128)

            for k in range(NCH):
                # xs[:, :, 0] = x (bf16, cast rides the DMA), xs[:, :, 1] =
                # softplus(x) via exp then ln(1+.) — the "+1" rides the Ln
                # activation bias. Per-row [2, 63] is contiguous so the
                # matmul lhsT slice is a single 126-wide free dim.
                xs = xpool.tile([128, R, 2, KM1], bf16, tag="xs")
                nc.gpsimd.dma_start(xs[:, :, 0, :], lg[:, k * R:(k + 1) * R, :])
                et = epool.tile([128, R, KM1], bf16, tag="et")
                nc.scalar.activation(et[:], xs[:, :, 0, :], Act.Exp)
                nc.scalar.activation(xs[:, :, 1, :], et[:], Act.Ln, bias=1.0)
                oh = opool.tile([128, R, 64], bf16, tag="oh")
                nc.vector.tensor_tensor(
                    oh[:],
                    tload[:, k * R:(k + 1) * R, None].to_broadcast([128, R, 64]),
                    iota_f[:][:, None, :].to_broadcast([128, R, 64]),
                    Alu.is_equal,
                )
                for rr in range(R):
                    nc.tensor.matmul(
                        psum[:],
                        xs[:, rr].rearrange("p a b -> p (a b)"),
                        oh[:, rr, :],
                        start=(k == 0 and rr == 0),
                        stop=(k == NCH - 1 and rr == R - 1),
                    )

            # n2 = per-partition count of targets >= 2 (for the ln2 term)
            n2scr = fpool.tile([128, RTOT], f32, tag="n2scr")
            n2 = fpool.tile([128, 1], f32, tag="n2")
            nc.vector.tensor_scalar(
                n2scr[:], tload[:], 2.0, 0.0, Alu.is_ge, Alu.max,
                accum_out=n2[:],
            )

            # weights shard sum of squares
            wtile = fpool.tile([128, WPER // 128], f32, tag="wts")
            nc.sync.dma_start(wtile[:], wts.ap().rearrange("(p r) -> p r", p=128))
            wscr = fpool.tile([128, WPER // 128], f32, tag="wscr")
            wacc = fpool.tile([128, 1], f32, tag="wacc")
            nc.vector.scalar_tensor_tensor(
                wscr[:], wtile[:], 0.0, wtile[:], Alu.add, Alu.mult,
                accum_out=wacc[:],
            )

            # deltas (row 0 already dropped host-side; zeros on cores 1-7)
            dtile = fpool.tile([1, 192], f32, tag="dt")
            nc.sync.dma_start(dtile[:], dls.ap().rearrange("(p r) -> p r", p=1))
            dscr = fpool.tile([1, 192], f32, tag="dscr")
            dacc = fpool.tile([1, 1], f32, tag="dacc")
            nc.vector.scalar_tensor_tensor(
                dscr[:], dtile[:], 0.0, dtile[:], Alu.add, Alu.mult,
                accum_out=dacc[:],
            )

            # cdot[j] = sum_c psum[j, c] * coef[j, c]
            pscr = fpool.tile([126, 64], f32, tag="pscr")
            cdot = fpool.tile([126, 1], f32, tag="cdot")
            nc.vector.scalar_tensor_tensor(
                pscr[:], psum[:], 0.0, coeft[:], Alu.add, Alu.mult,
                accum_out=cdot[:],
            )

            # comb = (n2*ln2 [+ cdot on rows 0..125]) / B + wacc*alpha/2
            comb = fpool.tile([128, 1], f32, tag="comb")
            nc.vector.tensor_scalar_mul(comb[:], n2[:], LN2)
            nc.vector.tensor_tensor(comb[0:126], comb[0:126], cdot[:], Alu.add)
            nc.vector.tensor_scalar_mul(comb[:], comb[:], 1.0 / B)
            nc.vector.scalar_tensor_tensor(
                comb[:], wacc[:], ALPHA / 2.0, comb[:], Alu.mult, Alu.add,
            )

            # cross-partition sum via matmul with ones, then add delta term
            psum2 = p2pool.tile([1, 1], f32)
            nc.tensor.matmul(psum2[:], comb[:], ones[:], start=True, stop=True)
            fin = fpool.tile([1, 1], f32, tag="fin")
            nc.vector.scalar_tensor_tensor(
                fin[:], dacc[:], BETA / 2.0, psum2[:], Alu.mult, Alu.add,
            )
            nc.sync.dma_start(out.ap(), fin[:])

    nc.compile()
    return nc


def _get_prog():
    global _PROG
    if _PROG is None:
        _PROG = _build()
    return _PROG


def _in_maps(logits, targets, weights, deltas):
    lg = np.ascontiguousarray(logits, dtype=np.float32)
    tf = np.ascontiguousarray(targets).astype(np.float32)
    wf = np.ascontiguousarray(weights, dtype=np.float32).reshape(-1)
    d0 = np.zeros(192, dtype=np.float32)
    d0[:189] = np.asarray(deltas, dtype=np.float32)[1:].reshape(-1)
    dz = np.zeros(192, dtype=np.float32)
    coef = _coef()
    in_maps = []
    for c in range(NCORES):
        in_maps.append({
            "logits": lg[c * BC:(c + 1) * BC],
            "targets": tf[c * BC:(c + 1) * BC],
            "wts": wf[c * WPER:(c + 1) * WPER],
            "dls": d0 if c == 0 else dz,
            "coef": coef,
        })
    return in_maps


def kernel(logits, targets, weights, deltas):
    from concourse.bass_utils import run_bass_kernel_spmd

    nc = _get_prog()
    in_maps = _in_maps(logits, targets, weights, deltas)
    res = run_bass_kernel_spmd(nc, in_maps, core_ids=list(range(NCORES)))
    total = sum(float(res.results[c]["out"][0, 0]) for c in range(NCORES))
    return np.array(total, dtype=np.float32)
